# revision 3
# baseline (speedup 1.0000x reference)
"""Trainium2 Bass kernel for nn_DomainProjectionLDP (moe_routing).

reference math:
    z   = einsum('bd,ndr->bnr', feats, V) * s * onehot(domain_ids)
    out = feats + einsum('bnr,ndr->bd', z, U)
    reg = mean-squared orthogonality penalty on U,V + nuclear penalty on s

Strategy (hardcoded, 8 NeuronCores):
  - data-parallel shard feats/domain_ids along batch (8192 rows/core);
    per-domain params replicated on every core.
  - host ships X^T [1024, 8192] per core so both matmuls contract on the
    partition dimension without on-device transposes; the device computes
    z^T = (V*s)^T X^T, masks it with (dom==n) via one fused
    scalar_tensor_tensor per j-chunk, then Af^T = U_flat z_m^T and
    out^T = X^T + Af^T.  Matmuls run in bf16 (fp32 accumulate); the
    residual add is exact fp32.
  - the regularizer is computed on-device with exact fp32 matmuls,
    replicated on every core (core 0's value is returned).
"""

import os
import sys

import numpy as np

sys.path.insert(0, "/opt/trn_rl_repo")

import ml_dtypes

B, D, ND, R = 65536, 1024, 8, 32
NCORES = 8
BC = B // NCORES          # 8192 rows per core
P = 128                   # partitions
NB = 512                  # b-tile width
NT = BC // NB             # 16 b-tiles per core
J = ND * R                # 256 low-rank columns across domains
DC = D // P               # 8 d-chunks
JC = J // P               # 2 j-chunks

_PROGRAM = None


def _program():
    """Build + lower the (SPMD, per-core) Bass program once."""
    global _PROGRAM
    if _PROGRAM is not None:
        return _PROGRAM

    import concourse.bacc as bacc
    import concourse.mybir as mybir
    from concourse.tile import TileContext

    f32 = mybir.dt.float32
    bf16 = mybir.dt.bfloat16
    Alu = mybir.AluOpType

    nc = bacc.Bacc("TRN2", target_bir_lowering=False, debug=False,
                   num_devices=NCORES)

    xtd = nc.dram_tensor("xtd", [D, BC], f32, kind="ExternalInput")
    domr = nc.dram_tensor("domr", [1, BC], bf16, kind="ExternalInput")
    vs = nc.dram_tensor("vs", [D, J], bf16, kind="ExternalInput")      # (V*s)[d, j]
    ujd = nc.dram_tensor("ujd", [J, D], bf16, kind="ExternalInput")    # U[j, d]
    udf = nc.dram_tensor("udf", [D, J], f32, kind="ExternalInput")     # U[d, j]
    vdf = nc.dram_tensor("vdf", [D, J], f32, kind="ExternalInput")     # V[d, j]
    srow = nc.dram_tensor("srow", [1, J], f32, kind="ExternalInput")
    eyer = nc.dram_tensor("eyer", [R, 2 * ND * R], f32, kind="ExternalInput")

    outt = nc.dram_tensor("outt", [D, BC], f32, kind="ExternalOutput")
    regv = nc.dram_tensor("regv", [1, 1], f32, kind="ExternalOutput")
    scratch = nc.dram_tensor("scratch", [R], f32)

    with TileContext(nc) as tc:
        with (
            tc.tile_pool(name="const", bufs=1) as cpool,
            tc.tile_pool(name="xt", bufs=16) as xpool,
            tc.tile_pool(name="xb", bufs=16) as xbpool,
            tc.tile_pool(name="zm", bufs=4) as zpool,
            tc.tile_pool(name="ot", bufs=16) as opool,
            tc.tile_pool(name="regs", bufs=1) as rpool,
            tc.tile_pool(name="psz", bufs=4, space="PSUM") as pszpool,
            tc.tile_pool(name="psa", bufs=3, space="PSUM") as psapool,
            tc.tile_pool(name="psr", bufs=1, space="PSUM") as psrpool,
        ):
            # ---- resident constants ----
            vs_sb = cpool.tile([P, DC, J], bf16)       # [d%128, dchunk, j]
            nc.sync.dma_start(out=vs_sb[:], in_=vs[:].rearrange("(c p) j -> p c j", p=P))
            ujd_sb = [cpool.tile([P, D], bf16, tag=f"ujd{m}", name=f"ujd_sb{m}")
                      for m in range(JC)]
            for m in range(JC):
                nc.sync.dma_start(out=ujd_sb[m][:], in_=ujd[m * P:(m + 1) * P, :])

            # dom broadcast [128, BC] via log-doubling SBUF->SBUF DMAs
            dom_b = cpool.tile([P, BC], bf16)
            nc.sync.dma_start(out=dom_b[0:1, :], in_=domr[:])
            p = 1
            while p < P:
                k = min(p, P - p)
                nc.sync.dma_start(out=dom_b[p:p + k, :], in_=dom_b[0:k, :])
                p += k

            # per-partition domain index for each j-chunk: n(j) = j // R
            nvec = [cpool.tile([P, 1], f32, tag=f"nv{m}", name=f"nvec{m}")
                    for m in range(JC)]
            for m in range(JC):
                for g in range(P // R):
                    nc.vector.memset(nvec[m][g * R:(g + 1) * R, :], float(m * (P // R) + g))

            # ---- main loop over b-tiles ----
            for t in range(NT):
                bsl = slice(t * NB, (t + 1) * NB)
                xt_t = []
                xb_t = []
                for dc in range(DC):
                    xt = xpool.tile([P, NB], f32, tag="xt")
                    nc.sync.dma_start(out=xt[:], in_=xtd[dc * P:(dc + 1) * P, bsl])
                    xb = xbpool.tile([P, NB], bf16, tag="xb")
                    nc.scalar.copy(out=xb[:], in_=xt[:])
                    xt_t.append(xt)
                    xb_t.append(xb)

                # MM1: z^T[jchunk m] = sum_dc (V*s)[dc,m]^T @ X^T[dc]
                zm_t = []
                for m in range(JC):
                    psz = pszpool.tile([P, NB], f32, tag="psz")
                    for dc in range(DC):
                        nc.tensor.matmul(
                            psz[:],
                            vs_sb[:, dc, m * P:(m + 1) * P],
                            xb_t[dc][:],
                            start=(dc == 0), stop=(dc == DC - 1),
                        )
                    # fused mask: zm = (dom == n(j)) * z
                    zm = zpool.tile([P, NB], bf16, tag="zm")
                    nc.vector.scalar_tensor_tensor(
                        out=zm[:], in0=dom_b[:, bsl], scalar=nvec[m][:],
                        in1=psz[:], op0=Alu.is_equal, op1=Alu.mult)
                    zm_t.append(zm)

                # MM2 + residual: out^T[dc] = X^T[dc] + sum_m U[jm,dc]^T @ zm[m]
                for dc in range(DC):
                    psa = psapool.tile([P, NB], f32, tag="psa")
                    for m in range(JC):
                        nc.tensor.matmul(
                            psa[:],
                            ujd_sb[m][:, dc * P:(dc + 1) * P],
                            zm_t[m][:],
                            start=(m == 0), stop=(m == JC - 1),
                        )
                    ot = opool.tile([P, NB], f32, tag="ot")
                    nc.vector.tensor_add(out=ot[:], in0=psa[:], in1=xt_t[dc][:])
                    nc.sync.dma_start(out=outt[dc * P:(dc + 1) * P, bsl], in_=ot[:])

            # ---- regularizer (exact fp32), replicated on every core ----
            udf_sb = rpool.tile([P, DC, J], f32, tag="udf")
            vdf_sb = rpool.tile([P, DC, J], f32, tag="vdf")
            nc.sync.dma_start(out=udf_sb[:], in_=udf[:].rearrange("(c p) j -> p c j", p=P))
            nc.sync.dma_start(out=vdf_sb[:], in_=vdf[:].rearrange("(c p) j -> p c j", p=P))
            eyer_sb = rpool.tile([R, 2 * ND * R], f32, tag="eyer")
            nc.sync.dma_start(out=eyer_sb[:], in_=eyer[:])
            srow_sb = rpool.tile([1, J], f32, tag="srow")
            nc.sync.dma_start(out=srow_sb[:], in_=srow[:])

            # gram matrices: pr[:, g*32:(g+1)*32] = W_n^T W_n  (16 groups)
            pr = psrpool.tile([R, 2 * ND * R], f32)
            for w, wsb in enumerate((udf_sb, vdf_sb)):
                for n in range(ND):
                    g = w * ND + n
                    for dc in range(DC):
                        col = wsb[:, dc, n * R:(n + 1) * R]
                        nc.tensor.matmul(
                            pr[:, g * R:(g + 1) * R], col, col,
                            start=(dc == 0), stop=(dc == DC - 1),
                        )
            # sum((gram - I)^2) over everything
            tt = rpool.tile([R, 2 * ND * R], f32, tag="tt")
            nc.vector.tensor_sub(out=tt[:], in0=pr[:], in1=eyer_sb[:])
            sq = rpool.tile([R, 2 * ND * R], f32, tag="sq")
            nc.vector.tensor_mul(out=sq[:], in0=tt[:], in1=tt[:])
            rs = rpool.tile([R, 1], f32, tag="rs")
            nc.vector.reduce_sum(rs[:], sq[:], axis=mybir.AxisListType.X)
            # cross-partition sum via DRAM round-trip reshape
            nc.sync.dma_start(out=scratch[:], in_=rs[:])
            row32 = rpool.tile([1, R], f32, tag="row32")
            nc.sync.dma_start(out=row32[:], in_=scratch[:].rearrange("(o r) -> o r", o=1))
            rtot = rpool.tile([1, 1], f32, tag="rtot")
            nc.vector.reduce_sum(rtot[:], row32[:], axis=mybir.AxisListType.X)
            # nuclear term: sum |s|
            stot = rpool.tile([1, 1], f32, tag="stot")
            nc.vector.tensor_reduce(stot[:], srow_sb[:], axis=mybir.AxisListType.X,
                                    op=Alu.add, apply_absolute_value=True)
            # reg = rtot/(R*R*ND) + stot*0.1/(R*ND)
            c1 = rpool.tile([1, 1], f32, tag="c1")
            nc.vector.tensor_scalar_mul(c1[:], rtot[:], 1.0 / (R * R * ND))
            regt = rpool.tile([1, 1], f32, tag="regt")
            nc.vector.scalar_tensor_tensor(
                out=regt[:], in0=stot[:], scalar=0.1 / (R * ND), in1=c1[:],
                op0=Alu.mult, op1=Alu.add)
            nc.sync.dma_start(out=regv[:], in_=regt[:])

    nc.compile()
    _PROGRAM = nc
    return nc


def _prep(feats, domain_ids, U, s, V):
    """Build the 8 per-core input maps (host-side shard + layout)."""
    feats = np.ascontiguousarray(np.asarray(feats), dtype=np.float32)
    dom = np.asarray(domain_ids).astype(np.float32).reshape(B)
    U = np.asarray(U, dtype=np.float32)
    s = np.asarray(s, dtype=np.float32)
    V = np.asarray(V, dtype=np.float32)

    bf = ml_dtypes.bfloat16
    vs_np = (V * s[:, None, :]).transpose(1, 0, 2).reshape(D, J)       # [d, j]
    ujd_np = U.transpose(0, 2, 1).reshape(J, D)                        # [j, d]
    udf_np = U.transpose(1, 0, 2).reshape(D, J)                        # [d, j]
    vdf_np = V.transpose(1, 0, 2).reshape(D, J)
    srow_np = s.reshape(1, J)
    eyer_np = np.tile(np.eye(R, dtype=np.float32), (1, 2 * ND))        # [32, 512]

    shared = {
        "vs": vs_np.astype(bf),
        "ujd": ujd_np.astype(bf),
        "udf": udf_np,
        "vdf": vdf_np,
        "srow": srow_np,
        "eyer": eyer_np,
    }
    in_maps = []
    for c in range(NCORES):
        rows = slice(c * BC, (c + 1) * BC)
        in_maps.append(dict(
            shared,
            xtd=np.ascontiguousarray(feats[rows].T),
            domr=dom[rows].reshape(1, BC).astype(bf),
        ))
    return in_maps


def _execute(in_maps, trace=False):
    from concourse.bass_utils import run_bass_kernel_spmd
    nc = _program()
    return run_bass_kernel_spmd(nc, in_maps, core_ids=list(range(NCORES)),
                                trace=trace)


def kernel(feats, domain_ids, U, s, V):
    in_maps = _prep(feats, domain_ids, U, s, V)
    res = _execute(in_maps, trace=False)
    out = np.empty((B, D), dtype=np.float32)
    for c in range(NCORES):
        out[c * BC:(c + 1) * BC, :] = res.results[c]["outt"].T
    reg = res.results[0]["regv"].reshape(1).astype(np.float32)
    return out, reg
